# revision 12
# baseline (speedup 1.0000x reference)
"""Trainium2 Bass kernel for nn_CorefDecoderMangoes (coref coarse-to-fine decoder).

Pipeline:
  1. Greedy non-crossing top-span selection (sequential NMS-style scan) on host.
  2. Device (8 NeuronCores, SPMD, rows of the K=1600 top spans sharded in
     balanced chunk pairs (c, 15-c) so per-core top-k column work is equal):
       src^T = W_fast @ top_emb^T   (GEMM1, fp32r)
       pair  = src @ top_emb^T      (GEMM2, fp32r)
       fast  = pair + toeplitz(dist prior + mask + mention scores)
       top-50 per row via iterated Max8/MaxIndex8/MatchReplace8; each core
       branches on partition id to run only the column width its rows need.
  3. Host scatters per-core shards back, applies the deterministic -1e9
     tie-pad corner (rows < 50), returns
     (top_span_indices, top_ant_idx, top_ant_scores).
"""
import os

import numpy as np

import concourse.bacc as bacc
import concourse.mybir as mybir
import concourse.tile as tile
from concourse.bass_utils import run_bass_kernel_spmd

# Problem constants (hardcoded per the harness contract)
C = 30000          # candidates
D = 1556           # span embedding dim
NW = 4096          # num words
K = 1600           # num top spans
A = 50             # max top antecedents
N_CORES = 8
M = K // N_CORES   # 200 rows per core
MP = 256           # padded per-core row count (fp32r wants moving dim >= 256)
NEG = np.float32(-1e9)
LOG2 = 0.6931471805599453

DCH = [128] * 12 + [20]          # D = 1556 -> 13 chunks
DOF = [sum(DCH[:i]) for i in range(len(DCH))]
NGS = [128, 512, 512, 404]       # wT column groups (small head so GEMM1 starts early)
NGO = [0, 128, 640, 1152]
NGC = [[0], [1, 2, 3, 4], [5, 6, 7, 8], [9, 10, 11, 12]]  # n-chunks per group
JSZ = 400                        # GEMM2 output free-dim chunk (PSUM bank: <=512 f32)
NJ = K // JSZ                    # 4
QROWS = 100                      # row chunk size (16 chunks)
MCH = [QROWS, QROWS]             # per-core row chunks (chunk c, chunk 15-c)
MOF = [0, QROWS]
R8 = 7                           # max8 rounds (56 >= 50)

LAST_RESULT = None               # stashed BassKernelResults (test.py reads exec_time_ns)
_COMPILED = {}


def _core_chunks(c):
    return (15 - c, c)


# ---------------------------------------------------------------- host scan --
def _extract_top_spans(scores, starts, ends):
    """Exact replica of reference.extract_top_spans (with early stop)."""
    order = np.argsort(-scores, kind="stable").astype(np.int32)
    s = starts[order].astype(np.int64)
    e = ends[order].astype(np.int64)
    s2me = np.full(NW, -1, np.int64)
    e2ms = np.full(NW, -1, np.int64)
    count = 0
    sel = np.zeros(C, bool)
    for i in range(C):
        si = int(s[i]); ei = int(e[i])
        cross = False
        if ei > si:
            if s2me[si + 1 : ei + 1].max() > ei:
                cross = True
            if not cross:
                w = e2ms[si:ei]
                if ((w >= 0) & (w < si)).any():
                    cross = True
        if (not cross) and count < K:
            sel[i] = True
            count += 1
            if s2me[si] < ei:
                s2me[si] = ei
            if e2ms[ei] < 0 or si < e2ms[ei]:
                e2ms[ei] = si
            if count == K:
                break
    n_sel = count
    sort_key = np.where(sel, s * NW + e, np.iinfo(np.int32).max)
    order2 = np.argsort(sort_key, kind="stable")
    cand_sorted = order[order2]
    ranks = np.arange(K)
    top = np.where(ranks < n_sel, cand_sorted[:K], cand_sorted[0])
    return top.astype(np.int32)


def _bucket_distance(d):
    logspace = np.floor(np.log(np.maximum(d, 1).astype(np.float32)) / LOG2).astype(np.int32) + 3
    idx = np.where(d <= 4, d, logspace)
    return np.clip(idx, 0, 9)


# ------------------------------------------------------------- device build --
def _build():
    f32 = mybir.dt.float32
    f32r = mybir.dt.float32r
    u16 = mybir.dt.uint16
    nc = bacc.Bacc("TRN2", target_bir_lowering=False, debug=False,
                   num_devices=N_CORES)

    embT = nc.dram_tensor("embT", [D, K], f32r, kind="ExternalInput")
    embM = nc.dram_tensor("embM", [D, MP], f32r, kind="ExternalInput")
    wT = nc.dram_tensor("wT", [D, D], f32r, kind="ExternalInput")
    bf = nc.dram_tensor("bf", [D, 1], f32, kind="ExternalInput")
    tz = nc.dram_tensor("tz", [M, K], f32, kind="ExternalInput")
    out_s = nc.dram_tensor("out_s", [M, 8 * R8], f32, kind="ExternalOutput")
    out_i = nc.dram_tensor("out_i", [M, 8 * R8], u16, kind="ExternalOutput")

    with tile.TileContext(nc) as tc:
        with (
            tc.tile_pool(name="persist", bufs=1) as pers,
            tc.tile_pool(name="wgroups", bufs=1) as wg,
            tc.tile_pool(name="small", bufs=1) as small,
        ):
            pid = nc.partition_id()

            # --- persistent loads (coalesced: 2 DMAs per tensor) -----------
            def folded_load(dram, width, dtype, tag, engine, dt_np_rows=D):
                """[D, width] DRAM -> big [128, 12*width] tile + [20, width] tail;
                returns per-d-chunk AP accessor."""
                big = pers.tile([128, 12 * width], dtype, tag=f"{tag}B",
                                name=f"{tag}B")
                engine.dma_start(
                    big[:].rearrange("p (c w) -> p c w", c=12),
                    dram[0:1536, 0:width].rearrange("(c p) w -> p c w", p=128))
                tail = pers.tile([20, width], dtype, tag=f"{tag}T", name=f"{tag}T")
                engine.dma_start(tail[:], dram[1536:1556, 0:width])
                return lambda kc: (big[:, kc * width:(kc + 1) * width]
                                   if kc < 12 else tail[:])
            embM_at = folded_load(embM, MP, f32r, "embM", nc.sync)
            embM_t = [embM_at(kc) for kc in range(13)]
            bf_at = folded_load(bf, 1, f32, "bf", nc.gpsimd)
            bf_t = [bf_at(kc) for kc in range(13)]
            tz_t = []
            for mc in range(2):
                t = pers.tile([MCH[mc], K], f32, tag=f"tz{mc}", name=f"tz{mc}")
                nc.gpsimd.dma_start(t[:], tz[MOF[mc]:MOF[mc] + MCH[mc], :])
                tz_t.append(t)
            embT_at = folded_load(embT, K, f32r, "embT", nc.scalar)
            embT_t = [embT_at(kc) for kc in range(13)]

            # --- GEMM1: srcT[n, m] = sum_d wT[d, n] * embM[d, m] + bf[n] ---
            srcT_t = [None] * 13
            with tc.tile_pool(name="psum1", bufs=3, space="PSUM") as pp1:
                for ng in range(len(NGS)):
                    gw = NGS[ng]
                    wbig = wg.tile([128, 12 * gw], f32r, tag="wtgB",
                                   name=f"wtgB{ng}")
                    nc.sync.dma_start(
                        wbig[:].rearrange("p (c w) -> p c w", c=12),
                        wT[0:1536, NGO[ng]:NGO[ng] + gw].rearrange(
                            "(c p) w -> p c w", p=128))
                    wtail = wg.tile([20, gw], f32r, tag="wtgT", name=f"wtgT{ng}")
                    nc.sync.dma_start(wtail[:],
                                      wT[1536:1556, NGO[ng]:NGO[ng] + gw])
                    wtg = [(wbig[:, dch * gw:(dch + 1) * gw] if dch < 12
                            else wtail[:]) for dch in range(13)]
                    for si, nch in enumerate(NGC[ng]):
                        nsz = DCH[nch]
                        ps = pp1.tile([nsz, MP], f32, tag="ps1", name=f"ps1_{nch}")
                        for dch in range(13):
                            nc.tensor.matmul(
                                ps[:], wtg[dch][:, si * 128:si * 128 + nsz],
                                embM_t[dch],
                                start=(dch == 0), stop=(dch == 12))
                        st = pers.tile([nsz, MP], f32r, tag=f"srcT{nch}",
                                       name=f"srcT{nch}")
                        nc.vector.tensor_scalar_add(st[:], ps[:], bf_t[nch])
                        srcT_t[nch] = st

            # --- GEMM2 + epilogue + per-core-width topk --------------------
            fast_t = [pers.tile([MCH[mc], K], f32, tag=f"fast{mc}", name=f"fast{mc}")
                      for mc in range(2)]
            slab_s = [pers.tile([MCH[mc], 8 * R8], f32, tag=f"ss{mc}", name=f"ss{mc}")
                      for mc in range(2)]
            slab_i = [pers.tile([MCH[mc], 8 * R8], u16, tag=f"si{mc}", name=f"si{mc}")
                      for mc in range(2)]

            def topk(mc, width):
                fa = fast_t[mc][:, :width]
                for r in range(R8):
                    sl = slice(8 * r, 8 * r + 8)
                    nc.vector.max(slab_s[mc][:, sl], fa)
                    nc.vector.max_index(slab_i[mc][:, sl], slab_s[mc][:, sl], fa)
                    if r != R8 - 1:
                        nc.vector.match_replace(fa, slab_s[mc][:, sl], fa, -3.0e38)

            with tc.tile_pool(name="psum2", bufs=2, space="PSUM") as pp2:
                for mc in range(2):
                    for jc in range(NJ):
                        ps2 = pp2.tile([MCH[mc], JSZ], f32, tag=f"ps2_{jc}",
                                       name=f"ps2_{mc}_{jc}")
                        for nch in range(13):
                            nc.tensor.matmul(
                                ps2[:],
                                srcT_t[nch][:, MOF[mc]:MOF[mc] + MCH[mc]],
                                embT_t[nch][:, jc * JSZ:(jc + 1) * JSZ],
                                start=(nch == 0), stop=(nch == 12))
                        nc.vector.tensor_add(
                            fast_t[mc][:, jc * JSZ:(jc + 1) * JSZ], ps2[:],
                            tz_t[mc][:, jc * JSZ:(jc + 1) * JSZ])
                    if mc == 0:
                        # big arm per core: runs while GEMM2 mc=1 is on the PE
                        for c in range(N_CORES):
                            with tc.If(pid == c):
                                topk(0, QROWS * (_core_chunks(c)[0] + 1))
                # small arm per core, after mc=1 epilogues
                for c in range(N_CORES):
                    with tc.If(pid == c):
                        topk(1, QROWS * (_core_chunks(c)[1] + 1))
                for mc in range(2):
                    nc.sync.dma_start(out_s[MOF[mc]:MOF[mc] + MCH[mc], :],
                                      slab_s[mc][:])
                    nc.sync.dma_start(out_i[MOF[mc]:MOF[mc] + MCH[mc], :],
                                      slab_i[mc][:])

    nc.compile()
    return nc


# ------------------------------------------------------------------ kernel --
def kernel(candidate_starts, candidate_ends, candidate_mention_scores,
           candidate_span_emb, W_fast, b_fast, emb_fast_distance, W_dist, b_dist,
           num_words, num_top_spans, max_top_antecedents):
    global LAST_RESULT
    scores = np.asarray(candidate_mention_scores, np.float32)
    starts = np.asarray(candidate_starts, np.int32)
    ends = np.asarray(candidate_ends, np.int32)
    emb = np.asarray(candidate_span_emb, np.float32)
    W_fast = np.asarray(W_fast, np.float32)
    b_fast = np.asarray(b_fast, np.float32)

    # 1. host scan
    top = _extract_top_spans(scores, starts, ends)
    ts = scores[top]                                   # [K]
    top_emb = emb[top]                                 # [K, D]

    # 2. device inputs
    embT_np = np.ascontiguousarray(top_emb.T)          # [D, K]
    wT_np = np.ascontiguousarray(W_fast.T)             # wT[d, n] = W_fast[n, d]
    bf_np = b_fast.reshape(D, 1)

    bucket_scores = (np.asarray(emb_fast_distance, np.float32) @
                     np.asarray(W_dist, np.float32) +
                     np.asarray(b_dist, np.float32)[0]).astype(np.float32)
    dd = np.arange(-(K - 1), K, dtype=np.int64)        # i - j
    vv = bucket_scores[_bucket_distance(np.maximum(dd, 0))].astype(np.float32)
    vv = vv + np.where(dd >= 1, np.float32(0), NEG)    # antecedent mask folded in

    jj = np.arange(K, dtype=np.int64)
    in_maps = []
    rows_by_core = []
    for c in range(N_CORES):
        qa, qb = _core_chunks(c)
        rows = np.concatenate([np.arange(QROWS * qa, QROWS * qa + QROWS),
                               np.arange(QROWS * qb, QROWS * qb + QROWS)])
        rows_by_core.append(rows)
        embM_c = np.zeros((D, MP), np.float32)
        embM_c[:, :M] = embT_np[:, rows]
        tz_c = (vv[(K - 1) + rows[:, None] - jj[None, :]] +
                ts[None, :] + ts[rows, None]).astype(np.float32)
        in_maps.append({
            "embT": embT_np,
            "embM": embM_c,
            "wT": wT_np,
            "bf": bf_np,
            "tz": np.ascontiguousarray(tz_c),
        })

    # 3. compile (cached) + run
    if "nc" not in _COMPILED:
        _COMPILED["nc"] = _build()
    nc = _COMPILED["nc"]
    try:                               # BASS_TRACE=1 without the optional axon
        import antenv.axon_hooks       # noqa: F401  hook module would crash in
    except ImportError:                # run_bass_kernel_spmd; degrade to no-trace
        import sys
        import types
        _m = types.ModuleType("antenv.axon_hooks")
        _m.get_axon_ntff_profile_hook = lambda: None
        _m.set_axon_ntff_profile_hook = lambda h: None
        sys.modules["antenv.axon_hooks"] = _m
    res = run_bass_kernel_spmd(nc, in_maps, core_ids=list(range(N_CORES)),
                               tmpdir=os.environ.get("COREF_TMPDIR"))
    LAST_RESULT = res

    # 4. assemble
    vals = np.empty((K, A), np.float32)
    idx = np.empty((K, A), np.int32)
    for c in range(N_CORES):
        vals[rows_by_core[c]] = res.results[c]["out_s"][:, :A]
        idx[rows_by_core[c]] = res.results[c]["out_i"][:, :A].astype(np.int32)
    for i in range(A):                                 # deterministic tie-pad corner
        ss = np.arange(i, A)
        idx[i, ss] = ss
        vals[i, ss] = NEG
    return top, idx, vals


# revision 13
# speedup vs baseline: 1.1497x; 1.1497x over previous
"""Trainium2 Bass kernel for nn_CorefDecoderMangoes (coref coarse-to-fine decoder).

Pipeline:
  1. Greedy non-crossing top-span selection (sequential NMS-style scan) on host.
  2. Device (8 NeuronCores, SPMD, rows of the K=1600 top spans sharded in
     balanced chunk pairs (c, 15-c) so per-core top-k column work is equal):
       src^T = W_fast @ top_emb^T   (GEMM1, fp32r)
       pair  = src @ top_emb^T      (GEMM2, fp32r)
       fast  = pair + toeplitz(dist prior + mask + mention scores)
       top-50 per row via iterated Max8/MaxIndex8/MatchReplace8; each core
       branches on partition id to run only the column width its rows need.
  3. Host scatters per-core shards back, applies the deterministic -1e9
     tie-pad corner (rows < 50), returns
     (top_span_indices, top_ant_idx, top_ant_scores).
"""
import os

import numpy as np

import concourse.bacc as bacc
import concourse.mybir as mybir
import concourse.tile as tile
from concourse.bass_utils import run_bass_kernel_spmd

# Problem constants (hardcoded per the harness contract)
C = 30000          # candidates
D = 1556           # span embedding dim
NW = 4096          # num words
K = 1600           # num top spans
A = 50             # max top antecedents
N_CORES = 8
M = K // N_CORES   # 200 rows per core
MP = 256           # padded per-core row count (fp32r wants moving dim >= 256)
NEG = np.float32(-1e9)
LOG2 = 0.6931471805599453

DCH = [128] * 12 + [20]          # D = 1556 -> 13 chunks
DOF = [sum(DCH[:i]) for i in range(len(DCH))]
NGS = [128, 512, 512, 404]       # wT column groups (small head so GEMM1 starts early)
NGO = [0, 128, 640, 1152]
NGC = [[0], [1, 2, 3, 4], [5, 6, 7, 8], [9, 10, 11, 12]]  # n-chunks per group
JSZ = 400                        # GEMM2 output free-dim chunk (PSUM bank: <=512 f32)
NJ = K // JSZ                    # 4
QROWS = 100                      # row chunk size (16 chunks)
MCH = [QROWS, QROWS]             # per-core row chunks (chunk c, chunk 15-c)
MOF = [0, QROWS]
R8 = 7                           # max8 rounds (56 >= 50)

LAST_RESULT = None               # stashed BassKernelResults (test.py reads exec_time_ns)
_COMPILED = {}


def _core_chunks(c):
    return (15 - c, c)


# ---------------------------------------------------------------- host scan --
def _extract_top_spans(scores, starts, ends):
    """Exact replica of reference.extract_top_spans (with early stop)."""
    order = np.argsort(-scores, kind="stable").astype(np.int32)
    s = starts[order].astype(np.int64)
    e = ends[order].astype(np.int64)
    s2me = np.full(NW, -1, np.int64)
    e2ms = np.full(NW, -1, np.int64)
    count = 0
    sel = np.zeros(C, bool)
    for i in range(C):
        si = int(s[i]); ei = int(e[i])
        cross = False
        if ei > si:
            if s2me[si + 1 : ei + 1].max() > ei:
                cross = True
            if not cross:
                w = e2ms[si:ei]
                if ((w >= 0) & (w < si)).any():
                    cross = True
        if (not cross) and count < K:
            sel[i] = True
            count += 1
            if s2me[si] < ei:
                s2me[si] = ei
            if e2ms[ei] < 0 or si < e2ms[ei]:
                e2ms[ei] = si
            if count == K:
                break
    n_sel = count
    sort_key = np.where(sel, s * NW + e, np.iinfo(np.int32).max)
    order2 = np.argsort(sort_key, kind="stable")
    cand_sorted = order[order2]
    ranks = np.arange(K)
    top = np.where(ranks < n_sel, cand_sorted[:K], cand_sorted[0])
    return top.astype(np.int32)


def _bucket_distance(d):
    logspace = np.floor(np.log(np.maximum(d, 1).astype(np.float32)) / LOG2).astype(np.int32) + 3
    idx = np.where(d <= 4, d, logspace)
    return np.clip(idx, 0, 9)


# ------------------------------------------------------------- device build --
def _build():
    f32 = mybir.dt.float32
    f32r = mybir.dt.float32r
    u16 = mybir.dt.uint16
    nc = bacc.Bacc("TRN2", target_bir_lowering=False, debug=False,
                   num_devices=N_CORES)

    embT = nc.dram_tensor("embT", [D, K], f32r, kind="ExternalInput")
    embM = nc.dram_tensor("embM", [D, MP], f32r, kind="ExternalInput")
    wT = nc.dram_tensor("wT", [D, D], f32r, kind="ExternalInput")
    bf = nc.dram_tensor("bf", [D, 1], f32, kind="ExternalInput")
    tz = nc.dram_tensor("tz", [M, K], f32, kind="ExternalInput")
    out_s = nc.dram_tensor("out_s", [M, 8 * R8], f32, kind="ExternalOutput")
    out_i = nc.dram_tensor("out_i", [M, 8 * R8], u16, kind="ExternalOutput")

    with tile.TileContext(nc) as tc:
        with (
            tc.tile_pool(name="persist", bufs=1) as pers,
            tc.tile_pool(name="wgroups", bufs=1) as wg,
            tc.tile_pool(name="small", bufs=1) as small,
        ):
            pid = nc.partition_id()

            # --- persistent loads ------------------------------------------
            # queue spread: GEMM1 operands on sync, embT on scalar, tz on gpsimd
            embM_t = []
            bf_t = []
            for kc in range(13):
                t = pers.tile([DCH[kc], MP], f32r, tag=f"embM{kc}", name=f"embM{kc}")
                nc.sync.dma_start(t[:], embM[DOF[kc]:DOF[kc] + DCH[kc], :])
                embM_t.append(t)
                b = small.tile([DCH[kc], 1], f32, tag=f"bf{kc}", name=f"bf{kc}")
                nc.gpsimd.dma_start(b[:], bf[DOF[kc]:DOF[kc] + DCH[kc], :])
                bf_t.append(b)
            tz_t = []
            for mc in range(2):
                t = pers.tile([MCH[mc], K], f32, tag=f"tz{mc}", name=f"tz{mc}")
                nc.gpsimd.dma_start(t[:], tz[MOF[mc]:MOF[mc] + MCH[mc], :])
                tz_t.append(t)
            # resident embT (GEMM2 rhs), 13 x [dsz, 1600]
            embT_t = []
            for kc in range(13):
                t = pers.tile([DCH[kc], K], f32r, tag=f"embT{kc}", name=f"embT{kc}")
                nc.scalar.dma_start(t[:], embT[DOF[kc]:DOF[kc] + DCH[kc], :])
                embT_t.append(t)

            # --- GEMM1: srcT[n, m] = sum_d wT[d, n] * embM[d, m] + bf[n] ---
            srcT_t = [None] * 13
            with tc.tile_pool(name="psum1", bufs=3, space="PSUM") as pp1:
                for ng in range(len(NGS)):
                    wtg = []
                    for dch in range(13):
                        w = wg.tile([DCH[dch], NGS[ng]], f32r, tag=f"wtg{dch}",
                                    name=f"wtg{ng}_{dch}")
                        nc.sync.dma_start(
                            w[:], wT[DOF[dch]:DOF[dch] + DCH[dch],
                                     NGO[ng]:NGO[ng] + NGS[ng]])
                        wtg.append(w)
                    for si, nch in enumerate(NGC[ng]):
                        nsz = DCH[nch]
                        ps = pp1.tile([nsz, MP], f32, tag="ps1", name=f"ps1_{nch}")
                        for dch in range(13):
                            nc.tensor.matmul(
                                ps[:], wtg[dch][:, si * 128:si * 128 + nsz],
                                embM_t[dch][:],
                                start=(dch == 0), stop=(dch == 12))
                        st = pers.tile([nsz, MP], f32r, tag=f"srcT{nch}",
                                       name=f"srcT{nch}")
                        nc.vector.tensor_scalar_add(st[:], ps[:], bf_t[nch][:])
                        srcT_t[nch] = st

            # --- GEMM2 + epilogue + per-core-width topk --------------------
            fast_t = [pers.tile([MCH[mc], K], f32, tag=f"fast{mc}", name=f"fast{mc}")
                      for mc in range(2)]
            slab_s = [pers.tile([MCH[mc], 8 * R8], f32, tag=f"ss{mc}", name=f"ss{mc}")
                      for mc in range(2)]
            slab_i = [pers.tile([MCH[mc], 8 * R8], u16, tag=f"si{mc}", name=f"si{mc}")
                      for mc in range(2)]

            def topk(mc, width):
                fa = fast_t[mc][:, :width]
                for r in range(R8):
                    sl = slice(8 * r, 8 * r + 8)
                    nc.vector.max(slab_s[mc][:, sl], fa)
                    nc.vector.max_index(slab_i[mc][:, sl], slab_s[mc][:, sl], fa)
                    if r != R8 - 1:
                        nc.vector.match_replace(fa, slab_s[mc][:, sl], fa, -3.0e38)

            with tc.tile_pool(name="psum2", bufs=2, space="PSUM") as pp2:
                for mc in range(2):
                    for jc in range(NJ):
                        ps2 = pp2.tile([MCH[mc], JSZ], f32, tag=f"ps2_{jc}",
                                       name=f"ps2_{mc}_{jc}")
                        for nch in range(13):
                            nc.tensor.matmul(
                                ps2[:],
                                srcT_t[nch][:, MOF[mc]:MOF[mc] + MCH[mc]],
                                embT_t[nch][:, jc * JSZ:(jc + 1) * JSZ],
                                start=(nch == 0), stop=(nch == 12))
                        nc.vector.tensor_add(
                            fast_t[mc][:, jc * JSZ:(jc + 1) * JSZ], ps2[:],
                            tz_t[mc][:, jc * JSZ:(jc + 1) * JSZ])
                    if mc == 0:
                        # big arm per core: runs while GEMM2 mc=1 is on the PE
                        for c in range(N_CORES):
                            with tc.If(pid == c):
                                topk(0, QROWS * (_core_chunks(c)[0] + 1))
                # small arm per core, after mc=1 epilogues
                for c in range(N_CORES):
                    with tc.If(pid == c):
                        topk(1, QROWS * (_core_chunks(c)[1] + 1))
                for mc in range(2):
                    nc.sync.dma_start(out_s[MOF[mc]:MOF[mc] + MCH[mc], :],
                                      slab_s[mc][:])
                    nc.sync.dma_start(out_i[MOF[mc]:MOF[mc] + MCH[mc], :],
                                      slab_i[mc][:])

    nc.compile()
    return nc


# ------------------------------------------------------------------ kernel --
def kernel(candidate_starts, candidate_ends, candidate_mention_scores,
           candidate_span_emb, W_fast, b_fast, emb_fast_distance, W_dist, b_dist,
           num_words, num_top_spans, max_top_antecedents):
    global LAST_RESULT
    scores = np.asarray(candidate_mention_scores, np.float32)
    starts = np.asarray(candidate_starts, np.int32)
    ends = np.asarray(candidate_ends, np.int32)
    emb = np.asarray(candidate_span_emb, np.float32)
    W_fast = np.asarray(W_fast, np.float32)
    b_fast = np.asarray(b_fast, np.float32)

    # 1. host scan
    top = _extract_top_spans(scores, starts, ends)
    ts = scores[top]                                   # [K]
    top_emb = emb[top]                                 # [K, D]

    # 2. device inputs
    embT_np = np.ascontiguousarray(top_emb.T)          # [D, K]
    wT_np = np.ascontiguousarray(W_fast.T)             # wT[d, n] = W_fast[n, d]
    bf_np = b_fast.reshape(D, 1)

    bucket_scores = (np.asarray(emb_fast_distance, np.float32) @
                     np.asarray(W_dist, np.float32) +
                     np.asarray(b_dist, np.float32)[0]).astype(np.float32)
    dd = np.arange(-(K - 1), K, dtype=np.int64)        # i - j
    vv = bucket_scores[_bucket_distance(np.maximum(dd, 0))].astype(np.float32)
    vv = vv + np.where(dd >= 1, np.float32(0), NEG)    # antecedent mask folded in

    jj = np.arange(K, dtype=np.int64)
    in_maps = []
    rows_by_core = []
    for c in range(N_CORES):
        qa, qb = _core_chunks(c)
        rows = np.concatenate([np.arange(QROWS * qa, QROWS * qa + QROWS),
                               np.arange(QROWS * qb, QROWS * qb + QROWS)])
        rows_by_core.append(rows)
        embM_c = np.zeros((D, MP), np.float32)
        embM_c[:, :M] = embT_np[:, rows]
        tz_c = (vv[(K - 1) + rows[:, None] - jj[None, :]] +
                ts[None, :] + ts[rows, None]).astype(np.float32)
        in_maps.append({
            "embT": embT_np,
            "embM": embM_c,
            "wT": wT_np,
            "bf": bf_np,
            "tz": np.ascontiguousarray(tz_c),
        })

    # 3. compile (cached) + run
    if "nc" not in _COMPILED:
        _COMPILED["nc"] = _build()
    nc = _COMPILED["nc"]
    try:                               # BASS_TRACE=1 without the optional axon
        import antenv.axon_hooks       # noqa: F401  hook module would crash in
    except ImportError:                # run_bass_kernel_spmd; degrade to no-trace
        import sys
        import types
        _m = types.ModuleType("antenv.axon_hooks")
        _m.get_axon_ntff_profile_hook = lambda: None
        _m.set_axon_ntff_profile_hook = lambda h: None
        sys.modules["antenv.axon_hooks"] = _m
    res = run_bass_kernel_spmd(nc, in_maps, core_ids=list(range(N_CORES)),
                               tmpdir=os.environ.get("COREF_TMPDIR"))
    LAST_RESULT = res

    # 4. assemble
    vals = np.empty((K, A), np.float32)
    idx = np.empty((K, A), np.int32)
    for c in range(N_CORES):
        vals[rows_by_core[c]] = res.results[c]["out_s"][:, :A]
        idx[rows_by_core[c]] = res.results[c]["out_i"][:, :A].astype(np.int32)
    for i in range(A):                                 # deterministic tie-pad corner
        ss = np.arange(i, A)
        idx[i, ss] = ss
        vals[i, ss] = NEG
    return top, idx, vals
